# revision 1
# baseline (speedup 1.0000x reference)
"""Trainium2 Bass kernel for nn_LEIterator (CG tensor-product iterator).

Layout/sharding: 8 cores = 2 sample-halves (128 samples on SBUF partitions)
x 4 k-groups (each core computes CG combination slots k in {2g, 2g+1}).
All gather indices are compile-time constants (seeded rng), so the per-core
gathers are done host-side into tiny pre-gathered input tensors; the device
program is identical on every core (pure SPMD) and consists of broadcast-AP
outer-product multiplies on the vector engine plus large contiguous output
DMAs (the kernel is output-write bound: ~43.5 MB/core).
"""

import numpy as np

import concourse.bass as bass
import concourse.mybir as mybir
from concourse.tile import TileContext
from concourse.vector_clock import ScopedClock
from concourse.bass_utils import run_bass_kernel_spmd


class _SplitDrainTC(TileContext):
    """TileContext whose kernel-tail drain spreads its semaphore waits over
    single-wait NOPs — this walrus codegen allows one sync wait per
    instruction (pseudo-direct DMA lowering), and the stock drain carries
    one wait per outstanding DMA lane."""

    def _drain_and_barrier(self, tick_clock, wait_clock):
        probe = self.nc.sync.nop(nofuse=True, hint="drain_waits")
        wait_clock.add_sem_waits(
            probe.ins, ScopedClock({None: tick_clock.global_clock})
        )
        si = probe.ins.sync_info
        waits = list(si.on_wait) if si is not None and si.on_wait else []
        probe.ins.sync_info = mybir.SyncInfo(on_wait=waits[:1], on_update=[])
        for w in waits[1:]:
            n = self.nc.sync.nop(nofuse=True, hint="drain_waits")
            n.ins.sync_info = mybir.SyncInfo(on_wait=[w], on_update=[])
        self.nc.sync.drain()
        self.nc.all_engine_barrier()
        popped = self.nc._tile_sem_poison_stack.pop()
        assert popped is self._sem_poison
        self.nc.clear_and_free_semaphores(list(self.sems.allocated().values()))
        self.nc.all_engine_barrier()

K = 8        # CG m-combinations kept per l_tuple
Q = 16       # radial channels
S = 256      # samples
L_MAX = 2
HALF = 128   # samples per core (S / 2 halves)
NU2_TUPLES = 6
NU3_TUPLES = 10
NU2_BLOCKS = NU2_TUPLES * 2   # per-core: 2 k-slots per tuple
NU3_BLOCKS = NU3_TUPLES * 2
QA0 = 0
QB0 = QA0 + NU2_BLOCKS * Q
PU0 = QB0 + NU2_BLOCKS * Q
PV0 = PU0 + NU3_BLOCKS * Q
PW0 = PV0 + NU3_BLOCKS * Q
INP_W = PW0 + NU3_BLOCKS * Q
ROWS2 = NU2_TUPLES * K * Q * Q          # 12288 rows in full output
ROWS3 = NU3_TUPLES * K * Q * Q * Q      # 327680
TOTAL_ROWS = ROWS2 + ROWS3              # 339968


def _build_structure():
    """Exact replica of reference._build_structure's rng call sequence."""
    rng = np.random.default_rng(0)
    t2 = []
    for l1 in range(L_MAX + 1):
        for l2 in range(l1, L_MAX + 1):
            ip = rng.integers(0, 2 * l1 + 1, K)
            i1 = rng.integers(0, 2 * l2 + 1, K)
            mult = (rng.random(K) + 0.5).astype(np.float32)
            t2.append(((l1, l2), ip, i1, mult))
    t3 = []
    for l1 in range(L_MAX + 1):
        for l2 in range(l1, L_MAX + 1):
            for l3 in range(l2, L_MAX + 1):
                ip = rng.integers(0, K, K)
                i1 = rng.integers(0, 2 * l3 + 1, K)
                mult = (rng.random(K) + 0.5).astype(np.float32)
                t3.append(((l1, l2, l3), ip, i1, mult))
    return t2, t3


_T2, _T3 = _build_structure()
_S2MAP = {lt: (ip, i1) for lt, ip, i1, _ in _T2}

_NC = None


def _build_program():
    f32 = mybir.dt.float32
    MULT = mybir.AluOpType.mult
    nc = bass.Bass("TRN2")

    inp = nc.dram_tensor("inp", [HALF, INP_W], f32, kind="ExternalInput")
    out2 = nc.dram_tensor("out2", [HALF, NU2_BLOCKS * Q * Q], f32, kind="ExternalOutput")
    out3 = nc.dram_tensor("out3", [HALF, NU3_BLOCKS * Q * Q * Q], f32, kind="ExternalOutput")

    with _SplitDrainTC(nc) as tc:
        with (
            tc.tile_pool(name="inp", bufs=1) as ipool,
            tc.tile_pool(name="big", bufs=2) as bpool,
        ):
            tinp = ipool.tile([HALF, INP_W], f32, tag="inp")
            nc.sync.dma_start(tinp[:], inp[:])
            tqa = tinp[:, QA0 : QA0 + NU2_BLOCKS * Q]
            tqb = tinp[:, QB0 : QB0 + NU2_BLOCKS * Q]
            tpu = tinp[:, PU0 : PU0 + NU3_BLOCKS * Q]
            tpv = tinp[:, PV0 : PV0 + NU3_BLOCKS * Q]
            tpw = tinp[:, PW0 : PW0 + NU3_BLOCKS * Q]

            # nu=3: 4 blocks per mega-tile. tmp_i = u (x) v [128,256] lives in
            # the head of the same pool tile as big_i = tmp_i (x) w [128,4096]
            # so the tmp TT (same-engine WAR is free) absorbs the slot-recycle
            # DMA WAR wait — the HW allows only ONE sync wait per instruction.
            # 7 total DMAs also keeps every DMAHW sem lane single-use, so no
            # DMA ever needs a lane-reuse wait on top of its producer wait.
            BPM = 4
            TMPW = BPM * Q * Q            # 1024 cols of tmp heads
            BIGW = BPM * Q * Q * Q        # 16384 cols of output blocks
            for m in range(NU3_BLOCKS // BPM):
                comb = bpool.tile([HALF, TMPW + BIGW], f32, tag="big")
                # tmp pairs: [p, 2, 16(a), 16(b)] = u (x) v, two blocks per TT
                for i in range(0, BPM, 2):
                    b = m * BPM + i
                    sl = slice(b * Q, (b + 2) * Q)
                    tmpv = comb[:, i * Q * Q : (i + 2) * Q * Q]
                    u = (
                        tpu[:, sl]
                        .rearrange("p (c a) -> p c a", a=Q)
                        .unsqueeze(3)
                        .broadcast_to([HALF, 2, Q, Q])
                    )
                    v = (
                        tpv[:, sl]
                        .rearrange("p (c b) -> p c b", b=Q)
                        .unsqueeze(2)
                        .broadcast_to([HALF, 2, Q, Q])
                    )
                    nc.vector.tensor_tensor(
                        tmpv.rearrange("p (c a b) -> p c a b", a=Q, b=Q), u, v, MULT
                    )
                # per-block big TTs: the 2-free-dim AP runs at full DVE rate
                # (a batched 3-free-dim stride-0 AP measured ~6% slower)
                for i in range(BPM):
                    b = m * BPM + i
                    sl = slice(b * Q, (b + 1) * Q)
                    tmpv = comb[:, i * Q * Q : (i + 1) * Q * Q]
                    bigv = comb[
                        :, TMPW + i * Q * Q * Q : TMPW + (i + 1) * Q * Q * Q
                    ]
                    t3d = tmpv.unsqueeze(2).broadcast_to([HALF, Q * Q, Q])
                    w = tpw[:, sl].unsqueeze(1).broadcast_to([HALF, Q * Q, Q])
                    nc.vector.tensor_tensor(
                        bigv.rearrange("p (f r) -> p f r", r=Q), t3d, w, MULT
                    )
                nc.sync.dma_start(
                    out3[:, m * BIGW : (m + 1) * BIGW], comb[:, TMPW:]
                )

            # nu=2 blocks: 3 TTs of 4 blocks into one staging tile, one DMA out
            t2s = ipool.tile([HALF, NU2_BLOCKS * Q * Q], f32, tag="o2")
            for b in range(0, NU2_BLOCKS, 4):
                sl = slice(b * Q, (b + 4) * Q)
                a = (
                    tqa[:, sl]
                    .rearrange("p (c a) -> p c a", a=Q)
                    .unsqueeze(3)
                    .broadcast_to([HALF, 4, Q, Q])
                )
                bb = (
                    tqb[:, sl]
                    .rearrange("p (c b) -> p c b", b=Q)
                    .unsqueeze(2)
                    .broadcast_to([HALF, 4, Q, Q])
                )
                o = t2s[:, b * Q * Q : (b + 4) * Q * Q].rearrange(
                    "p (c a b) -> p c a b", a=Q, b=Q
                )
                nc.vector.tensor_tensor(o, a, bb, MULT)
            nc.sync.dma_start(out2[:], t2s[:])
    return nc


def _get_nc():
    global _NC
    if _NC is None:
        _NC = _build_program()
    return _NC


def _make_in_maps(LE1):
    in_maps = []
    for c in range(8):
        h, g = divmod(c, 4)
        sl = slice(h * HALF, (h + 1) * HALF)
        buf = np.empty((HALF, INP_W), np.float32)
        qa = buf[:, QA0 : QA0 + NU2_BLOCKS * Q]
        qb = buf[:, QB0 : QB0 + NU2_BLOCKS * Q]
        pu = buf[:, PU0 : PU0 + NU3_BLOCKS * Q]
        pv = buf[:, PV0 : PV0 + NU3_BLOCKS * Q]
        pw = buf[:, PW0 : PW0 + NU3_BLOCKS * Q]
        for ti, ((l1, l2), ip, i1, mult) in enumerate(_T2):
            for j in range(2):
                k = 2 * g + j
                b = ti * 2 + j
                qa[:, b * Q : (b + 1) * Q] = LE1[l1][ip[k], :, sl].T
                qb[:, b * Q : (b + 1) * Q] = LE1[l2][i1[k], :, sl].T * mult[k]
        for ti, ((l1, l2, l3), ip3, i13, mult3) in enumerate(_T3):
            ip2, i12 = _S2MAP[(l1, l2)]
            for j in range(2):
                k = 2 * g + j
                b = ti * 2 + j
                kk = ip3[k]
                pu[:, b * Q : (b + 1) * Q] = LE1[l1][ip2[kk], :, sl].T
                pv[:, b * Q : (b + 1) * Q] = LE1[l2][i12[kk], :, sl].T
                pw[:, b * Q : (b + 1) * Q] = LE1[l3][i13[k], :, sl].T * mult3[k]
        in_maps.append({"inp": buf})
    return in_maps


LAST_RUN = None  # BassKernelResults of the most recent kernel() call (for test.py)
TRACE = False


def kernel(LE1_l0, LE1_l1, LE1_l2):
    global LAST_RUN
    LE1 = {
        0: np.ascontiguousarray(np.asarray(LE1_l0, dtype=np.float32)),
        1: np.ascontiguousarray(np.asarray(LE1_l1, dtype=np.float32)),
        2: np.ascontiguousarray(np.asarray(LE1_l2, dtype=np.float32)),
    }
    nc = _get_nc()
    in_maps = _make_in_maps(LE1)
    LAST_RUN = run_bass_kernel_spmd(
        nc, in_maps, core_ids=list(range(8)), trace=TRACE
    )
    res = LAST_RUN.results

    out = np.empty((TOTAL_ROWS, S), np.float32)
    for c in range(8):
        h, g = divmod(c, 4)
        cs = slice(h * HALF, (h + 1) * HALF)
        o2 = res[c]["out2"]
        o3 = res[c]["out3"]
        for ti in range(NU2_TUPLES):
            for j in range(2):
                k = 2 * g + j
                b = ti * 2 + j
                r0 = ti * (K * Q * Q) + k * Q * Q
                out[r0 : r0 + Q * Q, cs] = o2[:, b * Q * Q : (b + 1) * Q * Q].T
        for ti in range(NU3_TUPLES):
            for j in range(2):
                k = 2 * g + j
                b = ti * 2 + j
                w = Q * Q * Q
                r0 = ROWS2 + ti * (K * w) + k * w
                out[r0 : r0 + w, cs] = o3[:, b * w : (b + 1) * w].T
    return out



# revision 2
# speedup vs baseline: 1.6800x; 1.6800x over previous
"""Trainium2 Bass kernel for nn_LEIterator (CG tensor-product iterator).

Layout/sharding: 8 cores = 2 sample-halves (128 samples on SBUF partitions)
x 4 k-groups (each core computes CG combination slots k in {2g, 2g+1}).
All gather indices are compile-time constants (seeded rng), so the per-core
gathers are done host-side into tiny pre-gathered input tensors; the device
program is identical on every core (pure SPMD).

The kernel is output-write bound, so outputs are written as bf16 (rel-err
~1e-3, well under the 2e-2 gate) and widened to f32 on the host: 21.7 MB
of HBM writes per core against the ~358 GB/s per-core HBM limit (~61 us).
nu=3 expansion runs as per-partition-scalar tensor_scalar multiplies
(tmp[s, ab] * w[s, c], one instruction per c) whose dense bf16 APs hit the
DVE 4x perf mode, keeping DVE busy (~50 us) under the DMA floor. The
device-side nu3 block layout is [c, ab] (c outer) so every tensor_scalar
writes one dense 256-element run; the host unshuffle transposes back.
"""

import numpy as np

import concourse.bass as bass
import concourse.mybir as mybir
from concourse.tile import TileContext
from concourse.vector_clock import ScopedClock
from concourse.bass_utils import run_bass_kernel_spmd


class _SplitDrainTC(TileContext):
    """TileContext whose kernel-tail drain spreads its semaphore waits over
    single-wait NOPs — this walrus codegen allows one sync wait per
    instruction (pseudo-direct DMA lowering), and the stock drain carries
    one wait per outstanding DMA lane."""

    def _drain_and_barrier(self, tick_clock, wait_clock):
        probe = self.nc.sync.nop(nofuse=True, hint="drain_waits")
        wait_clock.add_sem_waits(
            probe.ins, ScopedClock({None: tick_clock.global_clock})
        )
        si = probe.ins.sync_info
        waits = list(si.on_wait) if si is not None and si.on_wait else []
        probe.ins.sync_info = mybir.SyncInfo(on_wait=waits[:1], on_update=[])
        for w in waits[1:]:
            n = self.nc.sync.nop(nofuse=True, hint="drain_waits")
            n.ins.sync_info = mybir.SyncInfo(on_wait=[w], on_update=[])
        self.nc.sync.drain()
        self.nc.all_engine_barrier()
        popped = self.nc._tile_sem_poison_stack.pop()
        assert popped is self._sem_poison
        self.nc.clear_and_free_semaphores(list(self.sems.allocated().values()))
        self.nc.all_engine_barrier()

K = 8        # CG m-combinations kept per l_tuple
Q = 16       # radial channels
S = 256      # samples
L_MAX = 2
HALF = 128   # samples per core (S / 2 halves)
NU2_TUPLES = 6
NU3_TUPLES = 10
NU2_BLOCKS = NU2_TUPLES * 2   # per-core: 2 k-slots per tuple
NU3_BLOCKS = NU3_TUPLES * 2
QA0 = 0
QB0 = QA0 + NU2_BLOCKS * Q
PU0 = QB0 + NU2_BLOCKS * Q
PV0 = PU0 + NU3_BLOCKS * Q
PW0 = PV0 + NU3_BLOCKS * Q
INP_W = PW0 + NU3_BLOCKS * Q
ROWS2 = NU2_TUPLES * K * Q * Q          # 12288 rows in full output
ROWS3 = NU3_TUPLES * K * Q * Q * Q      # 327680
TOTAL_ROWS = ROWS2 + ROWS3              # 339968
BW = Q * Q * Q                          # 4096 cols per nu3 block


def _build_structure():
    """Exact replica of reference._build_structure's rng call sequence."""
    rng = np.random.default_rng(0)
    t2 = []
    for l1 in range(L_MAX + 1):
        for l2 in range(l1, L_MAX + 1):
            ip = rng.integers(0, 2 * l1 + 1, K)
            i1 = rng.integers(0, 2 * l2 + 1, K)
            mult = (rng.random(K) + 0.5).astype(np.float32)
            t2.append(((l1, l2), ip, i1, mult))
    t3 = []
    for l1 in range(L_MAX + 1):
        for l2 in range(l1, L_MAX + 1):
            for l3 in range(l2, L_MAX + 1):
                ip = rng.integers(0, K, K)
                i1 = rng.integers(0, 2 * l3 + 1, K)
                mult = (rng.random(K) + 0.5).astype(np.float32)
                t3.append(((l1, l2, l3), ip, i1, mult))
    return t2, t3


_T2, _T3 = _build_structure()
_S2MAP = {lt: (ip, i1) for lt, ip, i1, _ in _T2}

_NC = None


def _build_program():
    f32 = mybir.dt.float32
    bf16 = mybir.dt.bfloat16
    MULT = mybir.AluOpType.mult
    nc = bass.Bass("TRN2")

    inp = nc.dram_tensor("inp", [HALF, INP_W], f32, kind="ExternalInput")
    out2 = nc.dram_tensor("out2", [HALF, NU2_BLOCKS * Q * Q], bf16, kind="ExternalOutput")
    out3 = nc.dram_tensor("out3", [HALF, NU3_BLOCKS * BW], bf16, kind="ExternalOutput")

    # out3 chunk DMA boundaries (cumulative block counts). 6 out3 DMAs +
    # out2 + input = 8 total, so every DMAHW sem lane stays single-use and
    # no DMA needs a lane-reuse wait on top of its producer wait.
    chunks = [2, 4, 8, 12, 16, 20]

    with _SplitDrainTC(nc) as tc:
        with tc.tile_pool(name="all", bufs=1) as pool:
            tinp = pool.tile([HALF, INP_W], f32, tag="inp")
            nc.sync.dma_start(tinp[:], inp[:])
            tqa = tinp[:, QA0 : QA0 + NU2_BLOCKS * Q]
            tqb = tinp[:, QB0 : QB0 + NU2_BLOCKS * Q]
            tpu = tinp[:, PU0 : PU0 + NU3_BLOCKS * Q]
            tpv = tinp[:, PV0 : PV0 + NU3_BLOCKS * Q]
            tpw = tinp[:, PW0 : PW0 + NU3_BLOCKS * Q]

            ttmp = pool.tile([HALF, NU3_BLOCKS * Q * Q], bf16, tag="tmp")
            t3s = pool.tile([HALF, NU3_BLOCKS * BW], bf16, tag="o3")
            t2s = pool.tile([HALF, NU2_BLOCKS * Q * Q], bf16, tag="o2")

            done = 0
            for pr in range(NU3_BLOCKS // 2):
                b0 = 2 * pr
                # tmp pair: [p, 2, 16(a), 16(b)] = u (x) v, two blocks per TT
                sl = slice(b0 * Q, (b0 + 2) * Q)
                tmpv = ttmp[:, b0 * Q * Q : (b0 + 2) * Q * Q]
                u = (
                    tpu[:, sl]
                    .rearrange("p (c a) -> p c a", a=Q)
                    .unsqueeze(3)
                    .broadcast_to([HALF, 2, Q, Q])
                )
                v = (
                    tpv[:, sl]
                    .rearrange("p (c b) -> p c b", b=Q)
                    .unsqueeze(2)
                    .broadcast_to([HALF, 2, Q, Q])
                )
                nc.vector.tensor_tensor(
                    tmpv.rearrange("p (c a b) -> p c a b", a=Q, b=Q), u, v, MULT
                )
                # nu3 blocks, [c, ab] device layout: 16 per-partition-scalar
                # multiplies per block, each a dense 256-wide bf16 run (4x).
                for b in (b0, b0 + 1):
                    tb = ttmp[:, b * Q * Q : (b + 1) * Q * Q]
                    for c in range(Q):
                        ov = t3s[:, b * BW + c * Q * Q : b * BW + (c + 1) * Q * Q]
                        nc.vector.tensor_scalar_mul(
                            ov, tb, tpw[:, b * Q + c : b * Q + c + 1]
                        )
                if b0 + 2 in chunks:
                    nc.sync.dma_start(
                        out3[:, done * BW : (b0 + 2) * BW],
                        t3s[:, done * BW : (b0 + 2) * BW],
                    )
                    done = b0 + 2

            # nu=2 blocks: 3 TTs of 4 blocks each, one DMA out
            for b in range(0, NU2_BLOCKS, 4):
                sl = slice(b * Q, (b + 4) * Q)
                a = (
                    tqa[:, sl]
                    .rearrange("p (c a) -> p c a", a=Q)
                    .unsqueeze(3)
                    .broadcast_to([HALF, 4, Q, Q])
                )
                bb = (
                    tqb[:, sl]
                    .rearrange("p (c b) -> p c b", b=Q)
                    .unsqueeze(2)
                    .broadcast_to([HALF, 4, Q, Q])
                )
                o = t2s[:, b * Q * Q : (b + 4) * Q * Q].rearrange(
                    "p (c a b) -> p c a b", a=Q, b=Q
                )
                nc.vector.tensor_tensor(o, a, bb, MULT)
            nc.sync.dma_start(out2[:], t2s[:])
    return nc


def _get_nc():
    global _NC
    if _NC is None:
        _NC = _build_program()
    return _NC


def _make_in_maps(LE1):
    in_maps = []
    for c in range(8):
        h, g = divmod(c, 4)
        sl = slice(h * HALF, (h + 1) * HALF)
        buf = np.empty((HALF, INP_W), np.float32)
        qa = buf[:, QA0 : QA0 + NU2_BLOCKS * Q]
        qb = buf[:, QB0 : QB0 + NU2_BLOCKS * Q]
        pu = buf[:, PU0 : PU0 + NU3_BLOCKS * Q]
        pv = buf[:, PV0 : PV0 + NU3_BLOCKS * Q]
        pw = buf[:, PW0 : PW0 + NU3_BLOCKS * Q]
        for ti, ((l1, l2), ip, i1, mult) in enumerate(_T2):
            for j in range(2):
                k = 2 * g + j
                b = ti * 2 + j
                qa[:, b * Q : (b + 1) * Q] = LE1[l1][ip[k], :, sl].T
                qb[:, b * Q : (b + 1) * Q] = LE1[l2][i1[k], :, sl].T * mult[k]
        for ti, ((l1, l2, l3), ip3, i13, mult3) in enumerate(_T3):
            ip2, i12 = _S2MAP[(l1, l2)]
            for j in range(2):
                k = 2 * g + j
                b = ti * 2 + j
                kk = ip3[k]
                pu[:, b * Q : (b + 1) * Q] = LE1[l1][ip2[kk], :, sl].T
                pv[:, b * Q : (b + 1) * Q] = LE1[l2][i12[kk], :, sl].T
                pw[:, b * Q : (b + 1) * Q] = LE1[l3][i13[k], :, sl].T * mult3[k]
        in_maps.append({"inp": buf})
    return in_maps


LAST_RUN = None  # BassKernelResults of the most recent kernel() call (for test.py)
TRACE = False


def kernel(LE1_l0, LE1_l1, LE1_l2):
    global LAST_RUN
    LE1 = {
        0: np.ascontiguousarray(np.asarray(LE1_l0, dtype=np.float32)),
        1: np.ascontiguousarray(np.asarray(LE1_l1, dtype=np.float32)),
        2: np.ascontiguousarray(np.asarray(LE1_l2, dtype=np.float32)),
    }
    nc = _get_nc()
    in_maps = _make_in_maps(LE1)
    LAST_RUN = run_bass_kernel_spmd(
        nc, in_maps, core_ids=list(range(8)), trace=TRACE
    )
    res = LAST_RUN.results

    out = np.empty((TOTAL_ROWS, S), np.float32)
    for core in range(8):
        h, g = divmod(core, 4)
        cs = slice(h * HALF, (h + 1) * HALF)
        # device outputs are bf16; widen exactly via bit shift
        o2 = np.asarray(res[core]["out2"])
        o2 = (o2.view(np.uint16).astype(np.uint32) << 16).view(np.float32)
        o3 = np.asarray(res[core]["out3"])
        o3 = (o3.view(np.uint16).astype(np.uint32) << 16).view(np.float32)
        for ti in range(NU2_TUPLES):
            for j in range(2):
                k = 2 * g + j
                b = ti * 2 + j
                r0 = ti * (K * Q * Q) + k * Q * Q
                out[r0 : r0 + Q * Q, cs] = o2[:, b * Q * Q : (b + 1) * Q * Q].T
        for ti in range(NU3_TUPLES):
            for j in range(2):
                k = 2 * g + j
                b = ti * 2 + j
                r0 = ROWS2 + ti * (K * BW) + k * BW
                # device block is [s, c, f]; reference rows are f*Q + c
                blk = o3[:, b * BW : (b + 1) * BW].reshape(HALF, Q, Q * Q)
                out[r0 : r0 + BW, cs] = blk.transpose(2, 1, 0).reshape(BW, HALF)
    return out
